# revision 30
# baseline (speedup 1.0000x reference)
"""Multi-head causal attention (B=2, S=2048, D=1024, H=16) on 8 TRN2 NeuronCores.

Sharding: batch x head-group.  Core i handles batch b = i//4 and head-group
hg = i%4 (4 heads = 256 projection columns).  Each core computes
  Q^T/K^T/V = proj(X_b) for its 256 columns, causal attention for its 4
  heads, and a partial output  ctx_slice @ Wo[256-row slice]  ->
  [2048, 1024] fp32 partial.  Host sums the 4 partials per batch and adds bo.

On-core algorithm (same math as the baseline, rescheduled):
  - scores TRANSPOSED: S^T[k, q] = K @ Q^T so softmax's k-reduction rides
    the PE ones-column trick; softmax without row-max (|s| < 70, fp32 exp
    cannot overflow); PV via lhsT = [1 | pad | V] packs the denominator
    into PSUM row 0.
  - q processed in 512-wide quarters: causal trim is exact at 128 cols
    (lo = 128(t-4qq)), so score/PV matmuls shrink toward the diagonal and
    no memsets are needed.
  - score matmuls for a head PAIR run CONCURRENTLY on the PE via row
    tiling (even head rows 0:63 / odd head rows 64:127, K=64 each); both
    land in one 2-bank PSUM tile so a single wide ACT exp covers the pair.
  - X^T built by fp32 PE transposes straight from the DMA'd X tiles (no
    pre-cast); the PSUM->SBUF detach does the fp16 convert.
  - software pipeline keeps the PE dense so the HAM clock gate stays at
    8/8 (2.4 GHz): block b interleaves attention(qq=b-1) with
    [transpose+V -> QK projection] for token group b; attention for the
    last quarter is split across the last two blocks with the out-
    projection as late filler.
  - engine balance: ACT = exp + weight converts + half the out-proj
    detaches; DVE = PSUM detaches, diagonal masks, normalize chain;
    GPSIMD = partition broadcasts only.
Pre-softmax chain in fp16, post-softmax in bf16.  Rel err ~3e-3.
"""

import numpy as np

import concourse.bass as bass
import concourse.mybir as mybir
import concourse.tile as tile
from concourse import bacc
from concourse.bass_utils import run_bass_kernel_spmd
from concourse.masks import make_identity

F32 = mybir.dt.float32
F16 = mybir.dt.float16
BF16 = mybir.dt.bfloat16
AF = mybir.ActivationFunctionType

B, S, D = 2, 2048, 1024
H, HD = 16, 64
NCORES = 8
CG = 256            # projection columns per core (4 heads)
HG_HEADS = 4        # heads per core
TOK_TILES = S // 128   # 16
D_CHUNKS = D // 128    # 8
QW = 512            # q quarter width
NQ = S // QW        # 4 quarters


def _build_program():
    nc = bacc.Bacc("TRN2", target_bir_lowering=False, debug=False)

    x_d = nc.dram_tensor("X", [S, D], F32, kind="ExternalInput").ap()
    wq_d = nc.dram_tensor("Wq", [D, CG], F32, kind="ExternalInput").ap()
    wk_d = nc.dram_tensor("Wk", [D, CG], F32, kind="ExternalInput").ap()
    wv_d = nc.dram_tensor("Wv", [D, CG], F32, kind="ExternalInput").ap()
    wo_d = nc.dram_tensor("Wo", [CG, D], F32, kind="ExternalInput").ap()
    out_d = nc.dram_tensor("out", [S, D], F32, kind="ExternalOutput").ap()

    with tile.TileContext(nc) as tc:
        _emit(nc, tc, x_d, wq_d, wk_d, wv_d, wo_d, out_d)
    nc.compile()
    return nc


def _emit(nc, tc, x_d, wq_d, wk_d, wv_d, wo_d, out_d):
    with (
        tc.sbuf_pool(name="persist", bufs=1) as pp,
        tc.sbuf_pool(name="work", bufs=1) as wp,
        tc.psum_pool(name="ps", bufs=1) as ap,
    ):
        # ---- persistent SBUF tensors
        xt = pp.tile([128, D_CHUNKS, S], F16, name="xt")        # X^T  [dval, dchunk, tok]
        qt = pp.tile([128, 2, S], F16, name="qt")               # Q^T  [parity*hd, pair, tok]
        kt = pp.tile([128, 2, S], F16, name="kt")
        vt = pp.tile([128, TOK_TILES, HG_HEADS * 128], BF16, name="vt")  # [1|pad|V]
        ctxT = pp.tile([128, 2, S], BF16, name="ctxT")
        wqh = pp.tile([128, D_CHUNKS, CG], F16, name="wqh")
        wkh = pp.tile([128, D_CHUNKS, CG], F16, name="wkh")
        wvh = pp.tile([128, D_CHUNKS, CG], F16, name="wvh")
        wob = pp.tile([128, 2, D], BF16, name="wob")
        ident = pp.tile([128, 128], F32, name="ident")
        cmask2 = pp.tile([128, 2, 128], BF16, name="cmask2")

        # ---- constants
        make_identity(nc, ident)
        # causal keep-mask for the diagonal 128x128 block of S^T[k, q]:
        # keep (1.0) where q >= k i.e. col >= partition.  One copy per
        # head-parity so a single DVE multiply masks the merged pb tile.
        nc.gpsimd.memset(cmask2, 1.0)
        for i in range(2):
            nc.gpsimd.affine_select(
                out=cmask2[:, i, :], in_=cmask2[:, i, :],
                compare_op=mybir.AluOpType.is_ge,
                fill=0.0, base=0, pattern=[[1, 128]], channel_multiplier=-1,
            )
        # ones column of [1|pad|V]: softmax denominator lands in PSUM row 0
        ones_cols = vt.rearrange("p t (h c) -> p t h c", h=HG_HEADS)[:, :, :, 0:1]
        nc.gpsimd.memset(ones_cols, 1.0)

        # ---- PE warm-up: dense matmuls on a memset tile while the first DMAs
        # land.  The HAM clock gate needs ~3.4us of continuous matmul activity
        # to lift the PE from 1.2 to 2.4 GHz; transposes don't count, so
        # without this the whole first block runs at half clock.
        wmw = wp.tile([128, 128], F16, name="wmw")
        wmr = wp.tile([128, 512], F16, name="wmr")
        nc.vector.memset(wmw, 1.0)
        nc.vector.memset(wmr, 0.0)
        for i in range(8):
            wmp = ap.tile([128, 512], F32, tag="fill", bufs=2, name="wmp")
            nc.tensor.matmul(wmp, lhsT=wmw, rhs=wmr, start=True, stop=True)

        # ---- X + weight loads: weights interleaved with the first X tiles so
        # Wv (needed by V at ~4us) isn't queued behind all 8MB of X.
        xs_tiles = [
            wp.tile([128, D], F32, tag="xs", bufs=6, name=f"xs{t}")
            for t in range(TOK_TILES)
        ]

        def load_weight(i):
            if i == 0:
                # chunked DMA+convert: each Wv d-chunk converts as it lands,
                # so the V(0) burst isn't gated on the whole 1MB + convert
                wstage = wp.tile([128, D_CHUNKS, CG], F32, tag="wstage", bufs=2)
                w_re = wv_d.rearrange("(dc p) c -> p dc c", p=128)
                for d in range(D_CHUNKS):
                    nc.sync.dma_start(wstage[:, d, :], w_re[:, d, :])
                    nc.scalar.copy(wvh[:, d, :], wstage[:, d, :])
            elif i < 3:
                w_dram, w_sb = ((None, None), (wq_d, wqh), (wk_d, wkh))[i]
                wstage = wp.tile([128, D_CHUNKS, CG], F32, tag="wstage", bufs=2)
                nc.sync.dma_start(
                    wstage, w_dram.rearrange("(dc p) c -> p dc c", p=128))
                nc.scalar.copy(w_sb, wstage)
            else:
                wostage = wp.tile([128, 2, D], F32, tag="wstage", bufs=2)
                nc.sync.dma_start(
                    wostage, wo_d.rearrange("(ct p) n -> p ct n", p=128))
                nc.scalar.copy(wob, wostage)

        # X tiles 0-2 first (the transpose stream consumes them from ~1.5us
        # and needs nothing else), then Wv so the V(0) burst at ~8us has its
        # weights, then the rest of the weights and X.
        for t in range(3):
            nc.sync.dma_start(xs_tiles[t], x_d[t * 128:(t + 1) * 128, :])
        load_weight(0)
        nc.sync.dma_start(xs_tiles[3], x_d[3 * 128:4 * 128, :])
        for i in range(1, 4):
            load_weight(i)
        for t in range(4, TOK_TILES):
            nc.sync.dma_start(xs_tiles[t], x_d[t * 128:(t + 1) * 128, :])

        vt_v = vt.rearrange("p t (h c) -> p t h c", h=HG_HEADS)

        def gen_qk(t4, cts):
            """QK projection for token group t4, coltiles in cts (coltile ==
            head pair).  Must follow gen_a(t4) in the PE FIFO (rhs needs all
            4 xt tiles of the group and transposes are PE work)."""
            for w_sb, dst in ((wqh, qt), (wkh, kt)):
                for ct in cts:
                    ps = ap.tile([128, 512], F32, tag="fill", bufs=2, name="ps")
                    for d in range(D_CHUNKS):
                        nc.tensor.matmul(
                            ps,
                            lhsT=w_sb[:, d, ct * 128:(ct + 1) * 128],
                            rhs=xt[:, d, t4 * 512:(t4 + 1) * 512],
                            start=(d == 0), stop=(d == D_CHUNKS - 1))
                        if d % 3 == 2:
                            yield
                    nc.vector.tensor_copy(dst[:, ct, t4 * 512:(t4 + 1) * 512], ps)
                    yield

        def gen_aqk(t4, cts=(0, 1)):
            """fp32 transposes + V-proj for token tiles 4*t4..4*t4+3, THEN
            the QK projection for that token group."""
            # xtp detaches ride on ACT in the early blocks (exp load is
            # light there) to keep the DVE queue from backing up the fill
            # slots the transposes need.
            det = nc.scalar.copy if t4 < 2 else nc.vector.tensor_copy
            for tt in range(4 * t4, 4 * t4 + 4):
                for dp in range(2):
                    xtp = ap.tile([128, 512], F32, tag="fill", bufs=2, name="xtp")
                    for dd in range(4):
                        d = dp * 4 + dd
                        nc.tensor.transpose(
                            xtp[:, dd * 128:(dd + 1) * 128],
                            xs_tiles[tt][:, d * 128:(d + 1) * 128], ident)
                    det(
                        xt[:, dp * 4:(dp + 1) * 4, tt * 128:(tt + 1) * 128],
                        xtp.rearrange("p (dd c) -> p dd c", dd=4))
                    yield
            for tt in range(4 * t4, 4 * t4 + 4):
                psv = ap.tile([128, 256], F32, tag="fill", bufs=2, name="psv")
                for d in range(D_CHUNKS):
                    nc.tensor.matmul(
                        psv,
                        lhsT=xt[:, d, tt * 128:(tt + 1) * 128],
                        rhs=wvh[:, d, :],
                        start=(d == 0), stop=(d == D_CHUNKS - 1))
                    if d == 3:
                        yield
                nc.vector.tensor_copy(
                    vt_v[:, tt, :, 64:128],
                    psv.rearrange("p (h c) -> p h c", h=HG_HEADS))
                yield
            yield from gen_qk(t4, cts)

        ctp_live = {}

        def gen_att(qq, pair, ts, te, norm):
            """Causal attention for q-quarter qq, head pair `pair`, k-tiles
            [ts, te).  Even/odd heads' score matmuls run concurrently via PE
            row tiling into one 2-bank sp tile; a single wide ACT exp covers
            both.  One-tile lookahead keeps scores(t) ahead of exp+PV(t-1)."""
            kmax = 4 * (qq + 1)
            if ts == 0:
                ctp_live[pair] = [
                    ap.tile([128, QW], F32, tag=f"ctp{par}", bufs=1,
                            name=f"ctp{par}")
                    for par in range(2)
                ]
            ctps = ctp_live[pair]
            prev = None
            for t in list(range(ts, te)) + [None]:
                cur = None
                if t is not None:
                    lo = max(0, 128 * (t - 4 * qq))
                    sp = ap.tile([128, 2, QW], F32, tag="sp", bufs=2, name="sp")
                    for par in range(2):
                        hr = par * 64
                        nc.tensor.matmul(
                            sp[:, par, lo:QW],
                            lhsT=kt[hr:hr + 64, pair, t * 128:(t + 1) * 128],
                            rhs=qt[hr:hr + 64, pair,
                                   qq * QW + lo:(qq + 1) * QW],
                            start=True, stop=True)
                    cur = (t, lo, sp)
                if prev is not None:
                    pt, plo, psp = prev
                    pb = wp.tile([128, 2, QW], BF16, tag="pb", bufs=3, name="pb")
                    nc.scalar.activation(
                        pb[:, :, plo:QW], psp[:, :, plo:QW], AF.Exp)
                    if pt >= 4 * qq:   # diagonal tile: mask boundary block
                        nc.vector.tensor_mul(
                            pb[:, :, plo:plo + 128], pb[:, :, plo:plo + 128],
                            cmask2)
                    for par in range(2):
                        h = 2 * pair + par
                        nc.tensor.matmul(
                            ctps[par][:, plo:QW],
                            lhsT=vt[:, pt, h * 128:(h + 1) * 128],
                            rhs=pb[:, par, plo:QW],
                            start=(pt == 0), stop=(pt == kmax - 1))
                prev = cur
                yield
            if norm:
                yield from gen_norm(qq, pair)

        def gen_norm(qq, pair, tail=False):
            # normalize: ctx^T = ctx~^T * (1/denom); denom is PSUM row 0.
            # The cst copy detaches the PSUM accumulator (frees the ctp slot
            # for the next pair) so the slow rec/broadcast/mul chain never
            # blocks the PE.  In the tail (nothing left to overlap) the
            # second parity's cst rides on ACT so the two chains pipeline
            # across engines.
            ctps = ctp_live[pair]
            for par in range(2):
                hr = par * 64
                cst = wp.tile([128, QW], F32, tag="cst", bufs=2, name="cst")
                if tail and par == 1:
                    nc.scalar.copy(cst, ctps[par])
                else:
                    nc.vector.tensor_copy(cst, ctps[par])
                rec = wp.tile([1, QW], F32, tag="rec", bufs=2, name="rec")
                # fast variant: ~18 correct bits, plenty for the softmax
                # denominator; halves the reciprocal stage of the chain
                nc.vector.reciprocal_approx_fast(rec, cst[0:1, :])
                bcr = wp.tile([128, QW], F32, tag="bcr", bufs=2, name="bcr")
                nc.gpsimd.partition_broadcast(bcr, rec, channels=128)
                nc.vector.tensor_mul(
                    ctxT[hr:hr + 64, pair, qq * QW:(qq + 1) * QW],
                    cst[64:128, :], bcr[64:128, :])
                yield

        def gen_att_q(qq):
            yield from gen_att(qq, 0, 0, 4 * (qq + 1), True)
            yield from gen_att(qq, 1, 0, 4 * (qq + 1), True)

        def gen_op(qq):
            """Out-projection for the 4 token tiles of quarter qq."""
            for tt in range(4 * qq, 4 * qq + 4):
                osb = wp.tile([128, D], F32, tag="osb", bufs=2, name="osb")
                for n in range(2):
                    pso = ap.tile([128, 512], F32, tag="fill", bufs=2,
                                  name="pso")
                    for x in range(2):
                        nc.tensor.matmul(
                            pso,
                            lhsT=ctxT[:, x, tt * 128:(tt + 1) * 128],
                            rhs=wob[:, x, n * 512:(n + 1) * 512],
                            start=(x == 0), stop=(x == 1))
                    # both detaches on DVE: ACT is the exp pacer in the
                    # blocks where out-proj runs as filler
                    nc.vector.tensor_copy(osb[:, n * 512:(n + 1) * 512], pso)
                    yield
                nc.sync.dma_start(out_d[tt * 128:(tt + 1) * 128, :], osb)
                yield

        def ileave(*gens):
            """Round-robin generator interleave (emission-order scheduler)."""
            gens = [iter(g) for g in gens]
            alive = [True] * len(gens)
            while any(alive):
                for i, g in enumerate(gens):
                    if alive[i]:
                        try:
                            next(g)
                            yield
                        except StopIteration:
                            alive[i] = False

        def chain(*gens):
            for g in gens:
                yield from g

        def run_gen(g):
            for _ in g:
                pass

        # ---- software pipeline.  attention(qq) only depends on gen_aqk
        # output from earlier blocks; the last quarter's attention is split
        # across the last blocks with out-proj and the deferred QK(3)/ct=1
        # projection as PE filler.  Out-proj matmuls are always emitted
        # BEFORE the normalize of the quarter running alongside them: a
        # ctxT write earlier in program order would serialize them behind
        # the slow normalize chain (conservative whole-tile dependency).
        run_gen(gen_aqk(0))
        run_gen(ileave(gen_att_q(0), gen_aqk(1)))
        run_gen(ileave(gen_att_q(1), gen_aqk(2)))
        run_gen(ileave(gen_att_q(2), chain(gen_aqk(3, cts=(0,)), gen_op(0))))
        run_gen(ileave(gen_att(3, 0, 0, 16, False),
                       ileave(gen_op(1), gen_op(2))))
        def gen_warm(n):
            # dummy matmuls: PE filler during the final normalize so HAM
            # doesn't re-throttle right before the last out-projection
            for i in range(n):
                wmp = ap.tile([128, 512], F32, tag="fill", bufs=2, name="wmp")
                nc.tensor.matmul(wmp, lhsT=wmw, rhs=wmr, start=True, stop=True)
                yield

        run_gen(ileave(gen_qk(3, cts=(1,)),
                       chain(gen_norm(3, 0), gen_att(3, 1, 0, 16, False))))
        run_gen(ileave(gen_norm(3, 1, tail=True), gen_warm(30)))
        run_gen(gen_op(3))


_PROGRAM = None


def _get_program():
    global _PROGRAM
    if _PROGRAM is None:
        _PROGRAM = _build_program()
    return _PROGRAM


def make_in_maps(X, Wq, Wk, Wv, Wo):
    X = np.asarray(X, dtype=np.float32)
    Wq = np.asarray(Wq, dtype=np.float32)
    Wk = np.asarray(Wk, dtype=np.float32)
    Wv = np.asarray(Wv, dtype=np.float32)
    Wo = np.asarray(Wo, dtype=np.float32)
    in_maps = []
    for core in range(NCORES):
        b, hg = core // 4, core % 4
        cs = slice(hg * CG, (hg + 1) * CG)
        in_maps.append({
            "X": np.ascontiguousarray(X[b]),
            "Wq": np.ascontiguousarray(Wq[:, cs]),
            "Wk": np.ascontiguousarray(Wk[:, cs]),
            "Wv": np.ascontiguousarray(Wv[:, cs]),
            "Wo": np.ascontiguousarray(Wo[cs, :]),
        })
    return in_maps


def combine_outputs(results, bo):
    bo = np.asarray(bo, dtype=np.float32)
    out = np.empty((B, S, D), dtype=np.float32)
    for b in range(B):
        acc = results[b * 4]["out"].copy()
        for hg in range(1, 4):
            acc += results[b * 4 + hg]["out"]
        out[b] = acc + bo[None, :]
    return out


def run(X, Wq, Wk, Wv, Wo, bo, **spmd_kwargs):
    nc = _get_program()
    in_maps = make_in_maps(X, Wq, Wk, Wv, Wo)
    res = run_bass_kernel_spmd(nc, in_maps, core_ids=list(range(NCORES)),
                               **spmd_kwargs)
    return combine_outputs(res.results, bo), res


def kernel(X, Wq, Wk, Wv, Wo, bo):
    out, _ = run(X, Wq, Wk, Wv, Wo, bo)
    return out


# revision 35
# speedup vs baseline: 1.0198x; 1.0198x over previous
"""Multi-head causal attention (B=2, S=2048, D=1024, H=16) on 8 TRN2 NeuronCores.

Sharding: batch x head-group.  Core i handles batch b = i//4 and head-group
hg = i%4 (4 heads = 256 projection columns).  Each core computes
  Q^T/K^T/V = proj(X_b) for its 256 columns, causal attention for its 4
  heads, and a partial output  ctx_slice @ Wo[256-row slice]  ->
  [2048, 1024] fp32 partial.  Host sums the 4 partials per batch and adds bo.

On-core algorithm (same math as the baseline, rescheduled):
  - scores TRANSPOSED: S^T[k, q] = K @ Q^T so softmax's k-reduction rides
    the PE ones-column trick; softmax without row-max (|s| < 70, fp32 exp
    cannot overflow); PV via lhsT = [1 | pad | V] packs the denominator
    into PSUM row 0.
  - q processed in 512-wide quarters: causal trim is exact at 128 cols
    (lo = 128(t-4qq)), so score/PV matmuls shrink toward the diagonal and
    no memsets are needed.
  - score matmuls for a head PAIR run CONCURRENTLY on the PE via row
    tiling (even head rows 0:63 / odd head rows 64:127, K=64 each); both
    land in one 2-bank PSUM tile so a single wide ACT exp covers the pair.
  - X^T built by fp32 PE transposes straight from the DMA'd X tiles (no
    pre-cast); the PSUM->SBUF detach does the fp16 convert.
  - software pipeline keeps the PE dense so the HAM clock gate stays at
    8/8 (2.4 GHz): block b interleaves attention(qq=b-1) with
    [transpose+V -> QK projection] for token group b; attention for the
    last quarter is split across the last two blocks with the out-
    projection as late filler.
  - engine balance: ACT = exp + weight converts + half the out-proj
    detaches; DVE = PSUM detaches, diagonal masks, normalize chain;
    GPSIMD = partition broadcasts only.
Pre-softmax chain in fp16, post-softmax in bf16.  Rel err ~3e-3.
"""

import numpy as np

import concourse.bass as bass
import concourse.mybir as mybir
import concourse.tile as tile
from concourse import bacc
from concourse.bass_utils import run_bass_kernel_spmd
from concourse.masks import make_identity

F32 = mybir.dt.float32
F16 = mybir.dt.float16
BF16 = mybir.dt.bfloat16
AF = mybir.ActivationFunctionType

B, S, D = 2, 2048, 1024
H, HD = 16, 64
NCORES = 8
CG = 256            # projection columns per core (4 heads)
HG_HEADS = 4        # heads per core
TOK_TILES = S // 128   # 16
D_CHUNKS = D // 128    # 8
QW = 512            # q quarter width
NQ = S // QW        # 4 quarters


def _build_program():
    nc = bacc.Bacc("TRN2", target_bir_lowering=False, debug=False)

    x_d = nc.dram_tensor("X", [S, D], F32, kind="ExternalInput").ap()
    wq_d = nc.dram_tensor("Wq", [D, CG], F32, kind="ExternalInput").ap()
    wk_d = nc.dram_tensor("Wk", [D, CG], F32, kind="ExternalInput").ap()
    wv_d = nc.dram_tensor("Wv", [D, CG], F32, kind="ExternalInput").ap()
    wo_d = nc.dram_tensor("Wo", [CG, D], F32, kind="ExternalInput").ap()
    out_d = nc.dram_tensor("out", [S, D], F32, kind="ExternalOutput").ap()

    with tile.TileContext(nc) as tc:
        _emit(nc, tc, x_d, wq_d, wk_d, wv_d, wo_d, out_d)
    nc.compile()
    return nc


def _emit(nc, tc, x_d, wq_d, wk_d, wv_d, wo_d, out_d):
    with (
        tc.sbuf_pool(name="persist", bufs=1) as pp,
        tc.sbuf_pool(name="work", bufs=1) as wp,
        tc.psum_pool(name="ps", bufs=1) as ap,
    ):
        # ---- persistent SBUF tensors
        xt = pp.tile([128, D_CHUNKS, S], F16, name="xt")        # X^T  [dval, dchunk, tok]
        qt = pp.tile([128, 2, S], F16, name="qt")               # Q^T  [parity*hd, pair, tok]
        kt = pp.tile([128, 2, S], F16, name="kt")
        vt = pp.tile([128, TOK_TILES, HG_HEADS * 128], BF16, name="vt")  # [1|pad|V]
        ctxT = pp.tile([128, 2, S], BF16, name="ctxT")
        wqh = pp.tile([128, D_CHUNKS, CG], F16, name="wqh")
        wkh = pp.tile([128, D_CHUNKS, CG], F16, name="wkh")
        wvh = pp.tile([128, D_CHUNKS, CG], F16, name="wvh")
        wob = pp.tile([128, 2, D], BF16, name="wob")
        ident = pp.tile([128, 128], F32, name="ident")
        F32R = mybir.dt.float32r
        cmask2 = pp.tile([128, 2, 128], BF16, name="cmask2")

        # ---- constants
        make_identity(nc, ident)
        # causal keep-mask for the diagonal 128x128 block of S^T[k, q]:
        # keep (1.0) where q >= k i.e. col >= partition.  One copy per
        # head-parity so a single DVE multiply masks the merged pb tile.
        nc.gpsimd.memset(cmask2, 1.0)
        for i in range(2):
            nc.gpsimd.affine_select(
                out=cmask2[:, i, :], in_=cmask2[:, i, :],
                compare_op=mybir.AluOpType.is_ge,
                fill=0.0, base=0, pattern=[[1, 128]], channel_multiplier=-1,
            )
        # ones column of [1|pad|V]: softmax denominator lands in PSUM row 0
        ones_cols = vt.rearrange("p t (h c) -> p t h c", h=HG_HEADS)[:, :, :, 0:1]
        nc.gpsimd.memset(ones_cols, 1.0)

        # ---- PE warm-up: dense matmuls on a memset tile while the first DMAs
        # land.  The HAM clock gate needs ~3.4us of continuous matmul activity
        # to lift the PE from 1.2 to 2.4 GHz; transposes don't count, so
        # without this the whole first block runs at half clock.
        wmw = wp.tile([128, 128], F16, name="wmw")
        wmr = wp.tile([128, 512], F16, name="wmr")
        nc.vector.memset(wmw, 1.0)
        nc.vector.memset(wmr, 0.0)
        for i in range(8):
            wmp = ap.tile([128, 512], F32, tag="fill", bufs=2, name="wmp")
            nc.tensor.matmul(wmp, lhsT=wmw, rhs=wmr, start=True, stop=True)

        # ---- X + weight loads: weights interleaved with the first X tiles so
        # Wv (needed by V at ~4us) isn't queued behind all 8MB of X.
        xs_tiles = [
            wp.tile([128, D], F32, tag="xs", bufs=6, name=f"xs{t}")
            for t in range(TOK_TILES)
        ]

        def load_weight(i):
            if i == 0:
                # chunked DMA+convert: each Wv d-chunk converts as it lands,
                # so the V(0) burst isn't gated on the whole 1MB + convert
                wstage = wp.tile([128, D_CHUNKS, CG], F32, tag="wstage", bufs=2)
                w_re = wv_d.rearrange("(dc p) c -> p dc c", p=128)
                for d in range(D_CHUNKS):
                    nc.sync.dma_start(wstage[:, d, :], w_re[:, d, :])
                    nc.scalar.copy(wvh[:, d, :], wstage[:, d, :])
            elif i < 3:
                w_dram, w_sb = ((None, None), (wq_d, wqh), (wk_d, wkh))[i]
                wstage = wp.tile([128, D_CHUNKS, CG], F32, tag="wstage", bufs=2)
                nc.sync.dma_start(
                    wstage, w_dram.rearrange("(dc p) c -> p dc c", p=128))
                nc.scalar.copy(w_sb, wstage)
            else:
                wostage = wp.tile([128, 2, D], F32, tag="wstage", bufs=2)
                nc.sync.dma_start(
                    wostage, wo_d.rearrange("(ct p) n -> p ct n", p=128))
                nc.scalar.copy(wob, wostage)

        # X tiles 0-2 first (the transpose stream consumes them from ~1.5us
        # and needs nothing else), then Wv so the V(0) burst at ~8us has its
        # weights, then the rest of the weights and X.
        for t in range(3):
            nc.sync.dma_start(xs_tiles[t], x_d[t * 128:(t + 1) * 128, :])
        load_weight(0)
        nc.sync.dma_start(xs_tiles[3], x_d[3 * 128:4 * 128, :])
        for i in range(1, 4):
            load_weight(i)
        for t in range(4, TOK_TILES):
            nc.sync.dma_start(xs_tiles[t], x_d[t * 128:(t + 1) * 128, :])

        vt_v = vt.rearrange("p t (h c) -> p t h c", h=HG_HEADS)

        def gen_qk(t4, cts):
            """QK projection for token group t4, coltiles in cts (coltile ==
            head pair).  Must follow gen_a(t4) in the PE FIFO (rhs needs all
            4 xt tiles of the group and transposes are PE work)."""
            for w_sb, dst in ((wqh, qt), (wkh, kt)):
                for ct in cts:
                    ps = ap.tile([128, 512], F32, tag="fill", bufs=2, name="ps")
                    for d in range(D_CHUNKS):
                        nc.tensor.matmul(
                            ps,
                            lhsT=w_sb[:, d, ct * 128:(ct + 1) * 128],
                            rhs=xt[:, d, t4 * 512:(t4 + 1) * 512],
                            start=(d == 0), stop=(d == D_CHUNKS - 1))
                        if d % 3 == 2:
                            yield
                    nc.vector.tensor_copy(dst[:, ct, t4 * 512:(t4 + 1) * 512], ps)
                    yield

        def gen_aqk(t4, cts=(0, 1)):
            """fp32 transposes + V-proj for token tiles 4*t4..4*t4+3, THEN
            the QK projection for that token group."""
            # xtp detaches ride on ACT in the early blocks (exp load is
            # light there) to keep the DVE queue from backing up the fill
            # slots the transposes need.
            det = nc.scalar.copy if t4 < 2 else nc.vector.tensor_copy
            for tt in range(4 * t4, 4 * t4 + 4):
                for dp in range(2):
                    xtp = ap.tile([128, 512], F32, tag="fill", bufs=2, name="xtp")
                    for dd in range(4):
                        d = dp * 4 + dd
                        nc.tensor.transpose(
                            xtp[:, dd * 128:(dd + 1) * 128],
                            xs_tiles[tt][:, d * 128:(d + 1) * 128], ident)
                    det(
                        xt[:, dp * 4:(dp + 1) * 4, tt * 128:(tt + 1) * 128],
                        xtp.rearrange("p (dd c) -> p dd c", dd=4))
                    yield
            for tt in range(4 * t4, 4 * t4 + 4):
                psv = ap.tile([128, 256], F32, tag="fill", bufs=2, name="psv")
                for d in range(D_CHUNKS):
                    nc.tensor.matmul(
                        psv,
                        lhsT=xt[:, d, tt * 128:(tt + 1) * 128],
                        rhs=wvh[:, d, :],
                        start=(d == 0), stop=(d == D_CHUNKS - 1))
                    if d == 3:
                        yield
                nc.vector.tensor_copy(
                    vt_v[:, tt, :, 64:128],
                    psv.rearrange("p (h c) -> p h c", h=HG_HEADS))
                yield
            yield from gen_qk(t4, cts)

        ctp_live = {}

        def gen_att(qq, pair, ts, te, norm):
            """Causal attention for q-quarter qq, head pair `pair`, k-tiles
            [ts, te).  Even/odd heads' score matmuls run concurrently via PE
            row tiling into one 2-bank sp tile; a single wide ACT exp covers
            both.  One-tile lookahead keeps scores(t) ahead of exp+PV(t-1)."""
            kmax = 4 * (qq + 1)
            if ts == 0:
                ctp_live[pair] = [
                    ap.tile([128, QW], F32, tag=f"ctp{par}", bufs=1,
                            name=f"ctp{par}")
                    for par in range(2)
                ]
            ctps = ctp_live[pair]
            prev = None
            for t in list(range(ts, te)) + [None]:
                cur = None
                if t is not None:
                    lo = max(0, 128 * (t - 4 * qq))
                    sp = ap.tile([128, 2, QW], F32, tag="sp", bufs=2, name="sp")
                    for par in range(2):
                        hr = par * 64
                        nc.tensor.matmul(
                            sp[:, par, lo:QW],
                            lhsT=kt[hr:hr + 64, pair, t * 128:(t + 1) * 128],
                            rhs=qt[hr:hr + 64, pair,
                                   qq * QW + lo:(qq + 1) * QW],
                            start=True, stop=True)
                    cur = (t, lo, sp)
                if prev is not None:
                    pt, plo, psp = prev
                    pb = wp.tile([128, 2, QW], BF16, tag="pb", bufs=3, name="pb")
                    nc.scalar.activation(
                        pb[:, :, plo:QW], psp[:, :, plo:QW], AF.Exp)
                    if pt >= 4 * qq:   # diagonal tile: mask boundary block
                        nc.vector.tensor_mul(
                            pb[:, :, plo:plo + 128], pb[:, :, plo:plo + 128],
                            cmask2)
                    for par in range(2):
                        h = 2 * pair + par
                        nc.tensor.matmul(
                            ctps[par][:, plo:QW],
                            lhsT=vt[:, pt, h * 128:(h + 1) * 128],
                            rhs=pb[:, par, plo:QW],
                            start=(pt == 0), stop=(pt == kmax - 1))
                prev = cur
                yield
            if norm:
                yield from gen_norm(qq, pair)

        def gen_norm(qq, pair, tail=False):
            # normalize: ctx^T = ctx~^T * (1/denom); denom is PSUM row 0.
            # The cst copy detaches the PSUM accumulator (frees the ctp slot
            # for the next pair) so the slow rec/broadcast/mul chain never
            # blocks the PE.  In the tail (nothing left to overlap) the
            # second parity's cst rides on ACT so the two chains pipeline
            # across engines.
            ctps = ctp_live[pair]
            for par in range(2):
                hr = par * 64
                cst = wp.tile([128, QW], F32, tag="cst", bufs=2, name="cst")
                if tail and par == 1:
                    nc.scalar.copy(cst, ctps[par])
                else:
                    nc.vector.tensor_copy(cst, ctps[par])
                rec = wp.tile([1, QW], F32, tag="rec", bufs=2, name="rec")
                # fast variant: ~18 correct bits, plenty for the softmax
                # denominator; halves the reciprocal stage of the chain
                nc.vector.reciprocal_approx_fast(rec, cst[0:1, :])
                bcr = wp.tile([128, QW], F32, tag="bcr", bufs=2, name="bcr")
                nc.gpsimd.partition_broadcast(bcr, rec, channels=128)
                nc.vector.tensor_mul(
                    ctxT[hr:hr + 64, pair, qq * QW:(qq + 1) * QW],
                    cst[64:128, :], bcr[64:128, :])
                yield

        def gen_att_q(qq):
            yield from gen_att(qq, 0, 0, 4 * (qq + 1), True)
            yield from gen_att(qq, 1, 0, 4 * (qq + 1), True)

        def gen_op(qq):
            """Out-projection for the 4 token tiles of quarter qq."""
            for tt in range(4 * qq, 4 * qq + 4):
                osb = wp.tile([128, D], F32, tag="osb", bufs=2, name="osb")
                for n in range(2):
                    pso = ap.tile([128, 512], F32, tag="fill", bufs=2,
                                  name="pso")
                    for x in range(2):
                        nc.tensor.matmul(
                            pso,
                            lhsT=ctxT[:, x, tt * 128:(tt + 1) * 128],
                            rhs=wob[:, x, n * 512:(n + 1) * 512],
                            start=(x == 0), stop=(x == 1))
                    if n == 0:
                        nc.scalar.copy(osb[:, n * 512:(n + 1) * 512], pso)
                    else:
                        nc.vector.tensor_copy(osb[:, n * 512:(n + 1) * 512], pso)
                    yield
                nc.sync.dma_start(out_d[tt * 128:(tt + 1) * 128, :], osb)
                yield

        def ileave(*gens):
            """Round-robin generator interleave (emission-order scheduler)."""
            gens = [iter(g) for g in gens]
            alive = [True] * len(gens)
            while any(alive):
                for i, g in enumerate(gens):
                    if alive[i]:
                        try:
                            next(g)
                            yield
                        except StopIteration:
                            alive[i] = False

        def chain(*gens):
            for g in gens:
                yield from g

        def run_gen(g):
            for _ in g:
                pass

        # ---- software pipeline.  attention(qq) only depends on gen_aqk
        # output from earlier blocks; the last quarter's attention is split
        # across the last blocks with out-proj and the deferred QK(3)/ct=1
        # projection as PE filler.  Out-proj matmuls are always emitted
        # BEFORE the normalize of the quarter running alongside them: a
        # ctxT write earlier in program order would serialize them behind
        # the slow normalize chain (conservative whole-tile dependency).
        run_gen(gen_aqk(0))
        run_gen(ileave(gen_att_q(0), gen_aqk(1)))
        run_gen(ileave(gen_att_q(1), gen_aqk(2)))
        run_gen(ileave(gen_att_q(2), chain(gen_aqk(3, cts=(0,)), gen_op(0))))
        run_gen(ileave(gen_att(3, 0, 0, 16, False),
                       ileave(gen_op(1), gen_op(2))))
        def gen_warm(n):
            # dummy matmuls: PE filler during the final normalize so HAM
            # doesn't re-throttle right before the last out-projection
            for i in range(n):
                wmp = ap.tile([128, 512], F32, tag="fill", bufs=2, name="wmp")
                nc.tensor.matmul(wmp, lhsT=wmw, rhs=wmr, start=True, stop=True)
                yield

        run_gen(ileave(gen_qk(3, cts=(1,)),
                       chain(gen_norm(3, 0), gen_att(3, 1, 0, 16, False))))
        run_gen(ileave(gen_norm(3, 1, tail=True), gen_warm(30)))
        run_gen(gen_op(3))


_PROGRAM = None


def _get_program():
    global _PROGRAM
    if _PROGRAM is None:
        _PROGRAM = _build_program()
    return _PROGRAM


def make_in_maps(X, Wq, Wk, Wv, Wo):
    X = np.asarray(X, dtype=np.float32)
    Wq = np.asarray(Wq, dtype=np.float32)
    Wk = np.asarray(Wk, dtype=np.float32)
    Wv = np.asarray(Wv, dtype=np.float32)
    Wo = np.asarray(Wo, dtype=np.float32)
    in_maps = []
    for core in range(NCORES):
        b, hg = core // 4, core % 4
        cs = slice(hg * CG, (hg + 1) * CG)
        in_maps.append({
            "X": np.ascontiguousarray(X[b]),
            "Wq": np.ascontiguousarray(Wq[:, cs]),
            "Wk": np.ascontiguousarray(Wk[:, cs]),
            "Wv": np.ascontiguousarray(Wv[:, cs]),
            "Wo": np.ascontiguousarray(Wo[cs, :]),
        })
    return in_maps


def combine_outputs(results, bo):
    bo = np.asarray(bo, dtype=np.float32)
    out = np.empty((B, S, D), dtype=np.float32)
    for b in range(B):
        acc = results[b * 4]["out"].copy()
        for hg in range(1, 4):
            acc += results[b * 4 + hg]["out"]
        out[b] = acc + bo[None, :]
    return out


def run(X, Wq, Wk, Wv, Wo, bo, **spmd_kwargs):
    nc = _get_program()
    in_maps = make_in_maps(X, Wq, Wk, Wv, Wo)
    res = run_bass_kernel_spmd(nc, in_maps, core_ids=list(range(NCORES)),
                               **spmd_kwargs)
    return combine_outputs(res.results, bo), res


def kernel(X, Wq, Wk, Wv, Wo, bo):
    out, _ = run(X, Wq, Wk, Wv, Wo, bo)
    return out
